# revision 12
# baseline (speedup 1.0000x reference)
"""Trainium2 Bass kernel for BinarizedLinear: y = x @ sign(W)^T.

Full-input contract: kernel(x, W) takes the unsharded inputs
(x: [8192, 4096] f32, W: [4096, 4096] f32) and returns y: [8192, 4096] f32.

Distribution: data-parallel over tokens. Each of the 8 NeuronCores gets a
[1024, 4096] token shard of x plus a full replica of W, computes
y_shard = x_shard @ sign(W)^T, and the shards are concatenated on the host.

Device kernel (per core):
  - sign(W) is computed on the Scalar (ACT) engine; the {-1, 0, +1} values
    are exact in fp16, so the matmul runs at the 16-bit TensorE rate
    (4x the fp32 rate). x is cast f32->f16 on the Vector engine (the only
    lossy step, ~2^-11 relative per element).
  - Matmuls contract over in_features (on SBUF partitions), accumulating
    32 k-tiles into PSUM in fp32. The first out-feature block uses all 8
    PSUM banks (its MM rate is DMA-matched while x streams in); later
    blocks use 4+4 so one group's accumulation overlaps the other's drain.
  - Host supplies transposed layouts (x^T per shard, W^T in o-block-major
    [OB, I, 512] order) so every DMA is a single linear transfer and the
    contraction dim lands on SBUF partitions with no on-device transposes.
  - The prologue interleaves W-block-0 and x tile loads per k-tile so the
    first matmuls start within a few microseconds.
"""

import numpy as np

TOKENS, IN_F, OUT_F = 8192, 4096, 4096
N_CORES = 8

LAST_RESULTS = None  # BassKernelResults of the most recent run (for profiling)
_NC_CACHE = {}


def _build_nc(T=TOKENS // N_CORES, I=IN_F, O=OUT_F, o_block=512, t_sub=4):
    """Build + compile the per-core Bass module.

    DRAM tensors (per core):
      xt:  [I, T] f32        -- x_shard^T
      wtb: [OB, I, o_block] f32 -- W^T, o-block-major
      y:   [T, O] f32
    """
    import concourse.mybir as mybir
    import concourse.tile as tile
    from concourse import bacc

    f32, f16 = mybir.dt.float32, mybir.dt.float16
    bf16 = mybir.dt.bfloat16

    P = 128
    KT = I // P          # k-tiles (contraction)
    OB = O // o_block    # output-feature blocks
    TT = T // P          # token tiles
    assert I % P == 0 and O % o_block == 0 and T % P == 0 and TT % t_sub == 0

    nc = bacc.Bacc(
        "TRN2", target_bir_lowering=False, debug=False, enable_asserts=False
    )
    xt = nc.dram_tensor("xt", [I, T], f32, kind="ExternalInput")
    # W^T travels as bf16: bf16 keeps f32's exponent range, so the cast
    # preserves sign exactly (no nonzero value rounds to zero); only sign(W)
    # is consumed, so this is a lossless encoding of the used information.
    wtb = nc.dram_tensor("wtb", [OB, I, o_block], bf16, kind="ExternalInput")
    y = nc.dram_tensor("y", [T, O], f32, kind="ExternalOutput")

    xt3 = xt.ap().rearrange("(k p) t -> k p t", p=P)       # [KT, 128, T]
    wt4 = wtb.ap().rearrange("b (k p) o -> b k p o", p=P)  # [OB, KT, 128, o_block]
    y3 = y.ap().rearrange("(t p) o -> t p o", p=P)         # [TT, 128, O]

    with tile.TileContext(nc) as tc:
        with (
            tc.tile_pool(name="xstage", bufs=3) as xstage_pool,
            tc.tile_pool(name="xres", bufs=KT) as xres_pool,
            tc.tile_pool(name="wstage", bufs=6) as wstage_pool,
            tc.tile_pool(name="wb", bufs=2 * KT) as wb_pool,
            tc.tile_pool(name="ystage", bufs=6) as ystage_pool,
            tc.tile_pool(name="psum", bufs=8, space="PSUM") as psum_pool,
        ):
            xf = [None] * KT
            wb = [None] * KT

            def load_x(k):
                st = xstage_pool.tile([P, T], f32, tag="xstage", name=f"xs_{k}")
                nc.sync.dma_start(st[:], xt3[k])
                xx = xres_pool.tile([P, T], f16, tag="xres", name=f"xf_{k}")
                nc.vector.tensor_copy(xx[:], st[:])
                xf[k] = xx

            def load_w(ob, k):
                st = wstage_pool.tile([P, o_block], bf16, tag="wstage",
                                      name=f"ws_{ob}_{k}")
                # W rides the Activation engine's HWDGE queue set so x (on
                # sync's) never queues behind it.
                nc.scalar.dma_start(st[:], wt4[ob, k])
                wbk = wb_pool.tile([P, o_block], f16, tag="wb", name=f"wb_{ob}_{k}")
                nc.scalar.sign(wbk[:], st[:])
                wb[k] = wbk

            def mm_group(ob, t0, nt):
                """Accumulate + drain output tiles for t-tiles t0..t0+nt-1."""
                osl = slice(ob * o_block, (ob + 1) * o_block)
                psums = [
                    psum_pool.tile([P, o_block], f32, tag="ps",
                                   name=f"ps_{ob}_{t0 + t}")
                    for t in range(nt)
                ]
                for k in range(KT):
                    for t in range(nt):
                        ti = t0 + t
                        nc.tensor.matmul(
                            psums[t][:],
                            xf[k][:, ti * P:(ti + 1) * P],  # lhsT [K, M]
                            wb[k][:],                        # rhs  [K, N]
                            start=(k == 0),
                            stop=(k == KT - 1),
                        )
                for t in range(nt):
                    ti = t0 + t
                    yt = ystage_pool.tile([P, o_block], f32, tag="ystage",
                                          name=f"yt_{ob}_{ti}")
                    nc.vector.tensor_copy(yt[:], psums[t][:])
                    nc.sync.dma_start(y3[ti][:, osl], yt[:])

            # Prologue: W block 0 and x interleaved per k-tile, then one
            # 8-bank MM group whose consumption rate matches DMA arrival.
            # (Block 0 is DMA-paced, so its early matmuls running at the
            # cold 1.2GHz clock warm the HAM gate for free.)
            for k in range(KT):
                load_w(0, k)
                load_x(k)
            assert TT <= 8
            mm_group(0, 0, TT)

            for ob in range(1, OB):
                for k in range(KT):
                    load_w(ob, k)
                for tg in range(TT // t_sub):
                    mm_group(ob, tg * t_sub, t_sub)

    nc.compile()
    return nc


def _get_nc(**kwargs):
    key = tuple(sorted(kwargs.items()))
    if key not in _NC_CACHE:
        _NC_CACHE[key] = _build_nc(**kwargs)
    return _NC_CACHE[key]


def _pack_w(W, o_block=512):
    """W [O, I] f32 -> o-block-major W^T [O//o_block, I, o_block], bf16.

    Only sign(W) is consumed on-device; the f32->bf16 cast preserves the
    sign of every value exactly (bf16 has f32's exponent range, so no
    nonzero f32 rounds to bf16 zero), making this a lossless wire encoding
    of the used information at half the DMA cost.
    """
    import ml_dtypes

    O, I = W.shape
    wt = W.T  # [I, O] view
    return np.ascontiguousarray(
        wt.reshape(I, O // o_block, o_block).transpose(1, 0, 2)
    ).astype(ml_dtypes.bfloat16)


def kernel(x, W):
    import os

    from concourse.bass_utils import run_bass_kernel_spmd

    global LAST_RESULTS

    # A stray BASS_TRACE in the environment would route run_bass_kernel_spmd
    # through the NTFF profiling hook, which needs antenv.axon_hooks; if
    # that module isn't importable here, neutralize tracing instead of
    # crashing.
    try:
        import antenv.axon_hooks  # noqa: F401
    except ImportError:
        os.environ.setdefault("BASS_NEVER_TRACE", "1")

    x = np.ascontiguousarray(np.asarray(x), dtype=np.float32)
    W = np.ascontiguousarray(np.asarray(W), dtype=np.float32)
    assert x.shape == (TOKENS, IN_F), x.shape
    assert W.shape == (OUT_F, IN_F), W.shape

    T = TOKENS // N_CORES
    nc = _get_nc()

    wtb = _pack_w(W)
    in_maps = [
        {"xt": np.ascontiguousarray(x[c * T:(c + 1) * T].T), "wtb": wtb}
        for c in range(N_CORES)
    ]

    res = run_bass_kernel_spmd(nc, in_maps, core_ids=list(range(N_CORES)))
    LAST_RESULTS = res
    return np.concatenate([r["y"] for r in res.results], axis=0)
